# revision 1
# baseline (speedup 1.0000x reference)
"""CrossModalAttention Trainium2 kernel v2 (8 NeuronCores, SPMD, no collectives).

Reference computation (B=4, S=2048, E=512, H=8, HD=64):
  Q = q_mod @ Wq + bq ; K = k_mod @ Wk + bk ; V = v_mod @ Wv + bv   (per head)
  scores = (Q K^T / sqrt(HD)) * modal_compat[h] ; attn = softmax(scores)
  out = (attn @ V) @ Wo + bo ; LayerNorm(out + q_mod) * gamma + beta

Sharding: core c handles batch b=c//2, query-rows half=c%2 (1024 rows each).
K/V are computed per batch on both cores of a pair (duplicated, no collectives).

v2 changes vs v1:
  - bf16 matmul operands (2 cols/cycle rhs streaming; fp32 PSUM accum).
  - score matmuls row-tiled: the two heads of a pair contract over disjoint
    PE row groups (partitions 0-63 / 64-127) and run concurrently.
  - attention processed per (head-pair, query-half): score PSUM tiles
    [128,1024] double-buffered (4 banks) + attended [65,512] x2 (2 banks)
    + V/out-proj pool (2 banks) = 8 banks, so score matmuls of tile t+1
    overlap the exp of tile t (ScalarE runs at ~100% duty).
  - attnV software-pipelined one kt behind exp.
  - V projection emitted just-in-time per key-tile inside the pair-0 loop.
  - softmax denominators: GPSIMD partition_broadcast + DVE
    reciprocal_approx_fast (replaces DRAM round-trip + 8-cyc/elem divide).
  - bv/bo folded on host into the residual (attn rows sum to 1 =>
    attended@Wo + bv@Wo + bo absorbed into xq_res); modal_compat/sqrt(HD)
    folded into Wq/bq as before.
  - LayerNorm uses fused scalar_tensor_tensor ops; the beta/gamma stage
    runs on GPSIMD to shorten the DVE tail.
"""
import sys
sys.path.insert(0, "/opt/trn_rl_repo")
import numpy as np

B, S, E, H = 4, 2048, 512, 8
HD = E // H
LN_EPS = 1e-5
N_CORES = 8
T = S // 2          # query rows per core
KT = S // 128       # key tiles (16)
TT = T // 128       # out row tiles per core (8)
NPAIR = H // 2      # head pairs (4)

_CACHE = {}


class _null_ctx:
    def __enter__(self):
        return None

    def __exit__(self, *a):
        return False


def build_nc(reps: int = 1, mmdt: str = "bf16", bcast: str = "gpsimd",
             ln_eng: str = "vector", recip: str = "fast", ablate: tuple = ()):
    import concourse.tile as tile
    from concourse import bacc, mybir
    import concourse.bass as bass

    f32 = mybir.dt.float32
    f32r = mybir.dt.float32r
    bf16 = mybir.dt.bfloat16
    mdt = {"bf16": bf16, "f32r": f32r}[mmdt]
    Exp = mybir.ActivationFunctionType.Exp
    Sqrt = mybir.ActivationFunctionType.Sqrt
    Alu = mybir.AluOpType

    nc = bacc.Bacc("TRN2", target_bir_lowering=False, debug=False,
                   enable_asserts=True, num_devices=N_CORES)
    dram = {}
    for name, shape, dt in [
        ("xqt", (E, T), mdt), ("xkt", (E, S), mdt), ("xvt", (E, S), mdt),
        ("wq", (E, E), mdt), ("wk", (E, E), mdt), ("wv", (E, E), mdt),
        ("wo", (E, E), mdt),
        ("bq", (E,), f32), ("bk", (E,), f32),
        ("xq_res", (T, E), f32), ("gamma", (E,), f32), ("beta", (E,), f32),
        ("vones", (128, 128), mdt),
    ]:
        dram[name] = nc.dram_tensor(name, shape, dt, kind="ExternalInput").ap()
    out_d = nc.dram_tensor("out", (T, E), f32, kind="ExternalOutput").ap()

    def pbcast(ap, parts):
        """AP view broadcasting partition dim (step 0) to `parts`."""
        return bass.AP(tensor=ap.tensor, offset=ap.offset,
                       ap=[[0, parts]] + list(ap.ap[1:]))

    with tile.TileContext(nc) as tc:
        with tc.tile_pool(name="consts", bufs=1) as consts, \
             tc.tile_pool(name="persist", bufs=1) as persist:
            # weights / biases / constants
            wq_sb = consts.tile([128, 4, E], mdt)
            wk_sb = consts.tile([128, 4, E], mdt)
            wv_sb = consts.tile([128, 4, E], mdt)
            wo_sb = consts.tile([128, 4, E], mdt)
            bq_sb = consts.tile([128, 4], f32)
            bk_sb = consts.tile([128, 4], f32)
            gamma_b = consts.tile([128, E], f32)
            beta_b = consts.tile([128, E], f32)
            eps_sb = consts.tile([128, 1], f32)

            # persistent activations
            if "smallexp" in ablate:
                esc_dummy = persist.tile([128, 1024], mdt)
                if mmdt == "bf16":
                    # bf16 1.0 == 0x3F80 == 16256
                    nc.gpsimd.memset(esc_dummy[:].bitcast(mybir.dt.int16), 16256)
                else:
                    nc.gpsimd.memset(esc_dummy[:].bitcast(mybir.dt.float32), 1.0)
            xqt_sb = persist.tile([128, 4, T], mdt)
            xkt_sb = persist.tile([128, 4, S], mdt)
            xvt_sb = persist.tile([128, 4, S], mdt)
            qt_sb = persist.tile([128, 4, T], mdt)     # Q.T feature-major
            kt_sb = persist.tile([128, 4, S], mdt)     # K.T feature-major
            v_sb = persist.tile([128, KT, H, HD + 1], mdt)  # V tokens + ones
            att_sb = persist.tile([128, 4, T], mdt)    # attended.T normalized
            xq_res_sb = persist.tile([128, TT, E], f32)

            nc.sync.dma_start(wq_sb, dram["wq"].rearrange("(k p) e -> p k e", p=128))
            nc.sync.dma_start(wk_sb, dram["wk"].rearrange("(k p) e -> p k e", p=128))
            nc.sync.dma_start(wv_sb, dram["wv"].rearrange("(k p) e -> p k e", p=128))
            nc.sync.dma_start(wo_sb, dram["wo"].rearrange("(k p) e -> p k e", p=128))
            nc.sync.dma_start(bq_sb, dram["bq"].rearrange("(m p) -> p m", p=128))
            nc.sync.dma_start(bk_sb, dram["bk"].rearrange("(m p) -> p m", p=128))
            nc.sync.dma_start(gamma_b, pbcast(dram["gamma"][None, :], 128))
            nc.sync.dma_start(beta_b, pbcast(dram["beta"][None, :], 128))
            nc.gpsimd.memset(eps_sb, LN_EPS)
            # ones column of V (col HD of each head group), from host consts
            nc.sync.dma_start(
                v_sb[:, :, :, HD:HD + 1],
                dram["vones"].rearrange("p (a b) -> p a b", a=KT)[:, :, :, None])

            def body():
                sc3 = "sc2bufs" not in ablate
                with tc.tile_pool(name="sc", bufs=3 if sc3 else 2,
                                  space="PSUM") as sc, \
                     tc.tile_pool(name="atp", bufs=1, space="PSUM") as atp, \
                     _null_ctx() if sc3 else tc.tile_pool(
                         name="vp", bufs=2, space="PSUM") as vp, \
                     tc.tile_pool(name="escp", bufs=4 if sc3 else 3) as escp, \
                     tc.tile_pool(name="dnp", bufs=2) as dnp, \
                     tc.tile_pool(name="dndp", bufs=2, space="DRAM") as dndp, \
                     tc.tile_pool(name="ln", bufs=2) as ln:

                    # ---- input DMAs (ordered by first use) ----
                    nc.sync.dma_start(
                        xqt_sb, dram["xqt"].rearrange("(k p) t -> p k t", p=128))
                    xk_r = dram["xkt"].rearrange("(k p) t -> p k t", p=128)
                    xv_r = dram["xvt"].rearrange("(k p) t -> p k t", p=128)
                    nc.sync.dma_start(xkt_sb[:, :, 0:1024], xk_r[:, :, 0:1024])
                    nc.sync.dma_start(xvt_sb[:, :, 0:1024], xv_r[:, :, 0:1024])
                    nc.sync.dma_start(xkt_sb[:, :, 1024:2048], xk_r[:, :, 1024:2048])
                    nc.sync.dma_start(xvt_sb[:, :, 1024:2048], xv_r[:, :, 1024:2048])
                    nc.sync.dma_start(
                        xq_res_sb, dram["xq_res"].rearrange("(t p) e -> p t e", p=128))

                    def proj_q(p):
                        q_ps = sc.tile([128, 1024], f32, tag="s")
                        for k in range(4):
                            for nn in range(2):
                                nc.tensor.matmul(
                                    q_ps[:, 512 * nn:512 * (nn + 1)],
                                    wq_sb[:, k, 128 * p:128 * (p + 1)],
                                    xqt_sb[:, k, 512 * nn:512 * (nn + 1)],
                                    start=(k == 0), stop=(k == 3))
                        nc.vector.tensor_scalar_add(
                            qt_sb[:, p, :], q_ps, scalar1=bq_sb[:, p:p + 1])

                    def proj_k(p, g2):
                        k_ps = sc.tile([128, 1024], f32, tag="s")
                        for k in range(4):
                            for gg in range(2):
                                c0 = 1024 * g2 + 512 * gg
                                nc.tensor.matmul(
                                    k_ps[:, 512 * gg:512 * (gg + 1)],
                                    wk_sb[:, k, 128 * p:128 * (p + 1)],
                                    xkt_sb[:, k, c0:c0 + 512],
                                    start=(k == 0), stop=(k == 3))
                        nc.vector.tensor_scalar_add(
                            kt_sb[:, p, 1024 * g2:1024 * (g2 + 1)], k_ps,
                            scalar1=bk_sb[:, p:p + 1])

                    def proj_v(tt):
                        if sc3:
                            v_full = sc.tile([128, 1024], f32, tag="s")
                            v_ps = v_full[:, 0:512]
                        else:
                            v_ps = vp.tile([128, 512], f32, tag="v")
                        for k in range(4):
                            nc.tensor.matmul(
                                v_ps, xvt_sb[:, k, 128 * tt:128 * (tt + 1)],
                                wv_sb[:, k, :], start=(k == 0), stop=(k == 3))
                        nc.vector.tensor_copy(
                            v_sb[:, tt, :, 0:HD],
                            v_ps.rearrange("p (h d) -> p h d", h=H))

                    # ---- startup projections for pair 0 ----
                    proj_q(0)
                    proj_k(0, 0)

                    def attention(p, n, hooks):
                        """Attention for head pair p, query half n."""
                        at_a = atp.tile([65, 512], f32, tag="atA")
                        at_b = atp.tile([65, 512], f32, tag="atB")
                        prev = None

                        def attnv(esc, kt):
                            if "fewattnv" in ablate and kt not in (0, KT - 1):
                                return
                            nc.tensor.matmul(
                                at_a, v_sb[:, kt, 2 * p, :], esc[:, 0:512],
                                start=(kt == 0), stop=(kt == KT - 1))
                            nc.tensor.matmul(
                                at_b, v_sb[:, kt, 2 * p + 1, :], esc[:, 512:1024],
                                start=(kt == 0), stop=(kt == KT - 1))

                        for kt in range(KT):
                            s_ps = sc.tile([128, 1024], f32, tag="s")
                            if "norowtile" in ablate:
                                nc.tensor.matmul(
                                    s_ps[:, 0:512],
                                    kt_sb[0:64, p, 128 * kt:128 * (kt + 1)],
                                    qt_sb[0:64, p, 512 * n:512 * (n + 1)],
                                    start=True, stop=True)
                                nc.tensor.matmul(
                                    s_ps[:, 512:1024],
                                    kt_sb[64:128, p, 128 * kt:128 * (kt + 1)],
                                    qt_sb[64:128, p, 512 * n:512 * (n + 1)],
                                    start=True, stop=True)
                            else:
                                nc.tensor.matmul(
                                    s_ps[:, 0:512],
                                    kt_sb[0:64, p, 128 * kt:128 * (kt + 1)],
                                    qt_sb[0:64, p, 512 * n:512 * (n + 1)],
                                    start=True, stop=True, tile_position=(0, 0))
                                nc.tensor.matmul(
                                    s_ps[:, 512:1024],
                                    kt_sb[64:128, p, 128 * kt:128 * (kt + 1)],
                                    qt_sb[64:128, p, 512 * n:512 * (n + 1)],
                                    start=True, stop=True, tile_position=(64, 0))
                            esc = escp.tile([128, 1024], mdt, tag="esc")
                            if "smallexp" in ablate:
                                nc.scalar.activation(
                                    out=esc[:, 0:64], in_=s_ps[:, 0:64], func=Exp)
                                esc = esc_dummy
                            else:
                                nc.scalar.activation(out=esc, in_=s_ps, func=Exp)
                            for h in hooks.get(kt, ()):
                                h()
                            if prev is not None:
                                attnv(*prev)
                            prev = (esc, kt)
                        attnv(*prev)

                        # normalize: att = at / den ; den in row 64 (ones col)
                        r2r = dnp.tile([128, 1024], f32, tag="r2r")
                        if "noden" in ablate:
                            nc.gpsimd.memset(r2r, 1.0)
                        else:
                            den = dnp.tile([1, 1024], f32, tag="den")
                            nc.vector.tensor_copy(den[0:1, 0:512], at_a[64:65, :])
                            nc.vector.tensor_copy(den[0:1, 512:1024], at_b[64:65, :])
                            r2 = dnp.tile([128, 1024], f32, tag="r2")
                            if bcast == "gpsimd":
                                nc.gpsimd.partition_broadcast(r2, den)
                            else:
                                den_d = dndp.tile([1, 1024], f32, tag="dend")
                                nc.sync.dma_start(den_d, den)
                                nc.sync.dma_start(r2, pbcast(den_d[0:1, :], 128))
                            if recip == "fast":
                                nc.vector.reciprocal_approx_fast(out=r2r, in_=r2)
                            else:
                                nc.vector.reciprocal(r2r, r2)
                        c0 = 512 * n
                        nc.vector.tensor_mul(
                            att_sb[0:64, p, c0:c0 + 512], at_a[0:64, :],
                            r2r[0:64, 0:512])
                        nc.vector.tensor_mul(
                            att_sb[64:128, p, c0:c0 + 512], at_b[0:64, :],
                            r2r[64:128, 512:1024])

                    def out_ln(tt):
                        if "noout" in ablate:
                            return
                        if sc3:
                            o_full = sc.tile([128, 1024], f32, tag="s")
                            o_ps = o_full[:, 0:512]
                        else:
                            o_ps = vp.tile([128, 512], f32, tag="v")
                        for k in range(4):
                            nc.tensor.matmul(
                                o_ps, att_sb[:, k, 128 * tt:128 * (tt + 1)],
                                wo_sb[:, k, :], start=(k == 0), stop=(k == 3))
                        x_sb = ln.tile([128, E], f32, tag="x")
                        nc.vector.tensor_add(x_sb, o_ps, xq_res_sb[:, tt, :])
                        stats = ln.tile([128, 6], f32, tag="st")
                        nc.vector.bn_stats(stats, x_sb)
                        mv = ln.tile([128, 2], f32, tag="mv")
                        nc.vector.bn_aggr(mv, stats)
                        sd = ln.tile([128, 1], f32, tag="sd")
                        nc.scalar.activation(out=sd, in_=mv[:, 1:2], func=Sqrt,
                                             bias=eps_sb)
                        rstd = ln.tile([128, 1], f32, tag="rs")
                        nc.vector.reciprocal(rstd, sd)
                        y1 = ln.tile([128, E], f32, tag="y1")
                        nc.vector.scalar_tensor_tensor(
                            out=y1, in0=x_sb, scalar=mv[:, 0:1], in1=gamma_b,
                            op0=Alu.subtract, op1=Alu.mult)
                        y = ln.tile([128, E], f32, tag="y")
                        eng = nc.gpsimd if ln_eng == "gpsimd" else nc.vector
                        eng.scalar_tensor_tensor(
                            out=y, in0=y1, scalar=rstd, in1=beta_b,
                            op0=Alu.mult, op1=Alu.add)
                        nc.sync.dma_start(out_d[128 * tt:128 * (tt + 1), :], y)

                    # ---- main loop: pairs x query-halves ----
                    for p in range(NPAIR):
                        for n in range(2):
                            hooks = {}
                            if p == 0 and n == 0:
                                # just-in-time V projection + rest of K(0)
                                for kt in range(KT):
                                    hooks[kt] = [(lambda t=kt: proj_v(t))]
                                hooks[6].append(lambda: proj_k(0, 1))
                            if n == 1 and p < NPAIR - 1:
                                hooks[8] = [lambda q=p + 1: proj_q(q),
                                            lambda q=p + 1: proj_k(q, 0)]
                            if n == 0 and p > 0 and p < NPAIR:
                                hooks[4] = [lambda q=p: proj_k(q, 1)]
                            attention(p, n, hooks)
                            if p == NPAIR - 1 and n == 0:
                                for tt in range(TT // 2):
                                    out_ln(tt)
                    for tt in range(TT // 2, TT):
                        out_ln(tt)

            if reps == 1:
                body()
            else:
                with tc.For_i(0, reps, 1):
                    body()

    nc.compile()
    return nc


def shard_inputs(query_modality, key_modality, value_modality, Wq, bq, Wk, bk,
                 Wv, bv, Wo, bo, modal_compat, ln_gamma, ln_beta):
    """Host-side prep: fold compat into Wq/bq, bv/bo into the residual,
    pre-transpose, cast matmul operands to bf16, shard per core."""
    import ml_dtypes
    f32 = np.float32
    bf = ml_dtypes.bfloat16
    q = np.asarray(query_modality, f32)
    k = np.asarray(key_modality, f32)
    v = np.asarray(value_modality, f32)
    compat = np.asarray(modal_compat, f32).reshape(H)
    scale = np.repeat(compat / np.sqrt(HD), HD).astype(f32)     # [E]
    wq_eff = (np.asarray(Wq, f32) * scale[None, :]).astype(f32)
    bq_eff = (np.asarray(bq, f32) * scale).astype(f32)
    # attn rows sum to 1 => (attended + bv) @ Wo + bo = attended @ Wo + fold
    res_fold = (np.asarray(bv, f32) @ np.asarray(Wo, f32)
                + np.asarray(bo, f32)).astype(f32)              # [E]

    shared = {
        "wq": wq_eff.astype(bf), "wk": np.asarray(Wk, f32).astype(bf),
        "wv": np.asarray(Wv, f32).astype(bf),
        "wo": np.asarray(Wo, f32).astype(bf),
        "bq": bq_eff, "bk": np.asarray(bk, f32),
        "gamma": np.asarray(ln_gamma, f32), "beta": np.asarray(ln_beta, f32),
        "vones": np.ones((128, 128), bf),
    }
    kt_cache = {}
    in_maps = []
    for c in range(N_CORES):
        b, half = c // 2, c % 2
        sl = slice(half * T, (half + 1) * T)
        if b not in kt_cache:
            kt_cache[b] = (np.ascontiguousarray(k[b].T).astype(bf),
                           np.ascontiguousarray(v[b].T).astype(bf))
        m = dict(shared)
        m["xqt"] = np.ascontiguousarray(q[b, sl, :].T).astype(bf)
        m["xkt"], m["xvt"] = kt_cache[b]
        m["xq_res"] = np.ascontiguousarray(q[b, sl, :] + res_fold[None, :])
        in_maps.append(m)
    return in_maps


def kernel(**inputs) -> np.ndarray:
    from concourse.bass_utils import run_bass_kernel_spmd

    if "nc" not in _CACHE:
        _CACHE["nc"] = build_nc(reps=1)
    nc = _CACHE["nc"]
    in_maps = shard_inputs(**inputs)
    res = run_bass_kernel_spmd(nc, in_maps, core_ids=list(range(N_CORES)))
    out = np.empty((B, S, E), np.float32)
    for c in range(N_CORES):
        b, half = c // 2, c % 2
        out[b, half * T:(half + 1) * T, :] = res.results[c]["out"]
    return out

